# revision 2
# baseline (speedup 1.0000x reference)
"""Causal self-attention kernel for 8 TRN2 NeuronCores.

Problem: B=4, T=2048, C=1024, H=16 heads, D=64 (fp32 in/out).
Sharding: 8 cores = 4 batch entries x 2 head-groups (8 heads each).

Key structure (vs the previous version): att@V runs in the FLIPPED
orientation — out[tq=128 partitions, 65 free] with P^T chunks as the
stationary operand and [v|ones] as the moving operand. The cost model
charges matmuls by output free size only, so this halves att@V's PE time
(65 per (tq-chunk, tk-chunk, head) instead of 512 per (tk-chunk, head)).
It also lands the softmax denominator on the partition axis, collapsing
normalization to one reciprocal + one broadcast-multiply per group.

psY accumulators are single-bank PSUM tiles shared by both heads; since
a matmul with start=True zeroes the entire 2KB bank (verified on HW),
the accumulators are instead zeroed by a DVE memset and all att@V
matmuls use start=False.

yT2 (proj stationary, [head-dim, t] layout) is rebuilt from the
normalized [tq, head-dim] tiles with PE transpose matmuls (identity
moving operand, 128 rows each) into ps_util banks bitcast to bf16.

Engine budget (cost model): PE ~200us (bottleneck), ACT exp ~149us,
DVE (memsets, qkv copies, normalization) ~63us, Pool (psY/psT copies,
causal masks) ~51us.
"""

import numpy as np
import ml_dtypes
import sys

sys.path.insert(0, "/opt/trn_rl_repo")

import concourse.bass as bass
import concourse.mybir as mybir
import concourse.tile as tile
from concourse import bacc
from concourse.bass_utils import run_bass_kernel_spmd

BF = mybir.dt.bfloat16
F32 = mybir.dt.float32
AF = mybir.ActivationFunctionType

B, T, C = 4, 2048, 1024
H, D = 16, 64
N_CORES = 8
HEADS_PER_CORE = 8          # 4 pairs
PAIRS = 4
TC = T // 128               # 16 t-chunks of 128
TG = T // 512               # 4 t-groups of 512
CT = C // 128               # 8 contraction tiles

_compiled = None


def _build():
    nc = bacc.Bacc("TRN2", target_bir_lowering=False)

    xT = nc.declare_dram_parameter("xT", [C, T], BF, isOutput=False)
    wq = nc.declare_dram_parameter("wq", [C, 512], BF, isOutput=False)
    wk = nc.declare_dram_parameter("wk", [C, 512], BF, isOutput=False)
    wv = nc.declare_dram_parameter("wv", [C, 512], BF, isOutput=False)
    wp = nc.declare_dram_parameter("wp", [512, C], BF, isOutput=False)
    tri = nc.declare_dram_parameter("tri", [128, 128], BF, isOutput=False)
    ident = nc.declare_dram_parameter("ident", [128, 128], BF, isOutput=False)
    y = nc.declare_dram_parameter("y", [T, C], F32, isOutput=True)

    with tile.TileContext(nc) as tc:
        with (
            tc.tile_pool(name="const", bufs=1) as cpool,
            tc.tile_pool(name="small", bufs=2) as spool,
            tc.tile_pool(name="ps_s", bufs=2, space="PSUM") as ps_s,
            tc.tile_pool(name="ps_y", bufs=2, space="PSUM") as ps_y,
            tc.tile_pool(name="ps_u", bufs=2, space="PSUM") as ps_u,
        ):
            # ---------------- constant loads ----------------
            xT_t = cpool.tile([128, CT, T], BF, name="xT_t", tag="xT_t")
            xT_r = xT.ap().rearrange("(ct p) t -> p ct t", p=128)
            wv_t = cpool.tile([128, CT, 512], BF, name="wv_t", tag="wv_t")
            wv_r = wv.ap().rearrange("(ct p) d -> p ct d", p=128)
            wq_t = cpool.tile([128, CT, 512], BF, name="wq_t", tag="wq_t")
            wq_r = wq.ap().rearrange("(ct p) d -> p ct d", p=128)
            wk_t = cpool.tile([128, CT, 512], BF, name="wk_t", tag="wk_t")
            wk_r = wk.ap().rearrange("(ct p) d -> p ct d", p=128)
            for ci in range(CT):
                nc.sync.dma_start(wv_t[:, ci], wv_r[:, ci])
                nc.sync.dma_start(xT_t[:, ci], xT_r[:, ci])
            for ci in range(CT):
                nc.sync.dma_start(wq_t[:, ci], wq_r[:, ci])
                nc.sync.dma_start(wk_t[:, ci], wk_r[:, ci])
            wp_t = cpool.tile([128, PAIRS, C], BF, name="wp_t", tag="wp_t")
            nc.sync.dma_start(wp_t[:], wp.ap().rearrange("(pr p) co -> p pr co", p=128))
            tri_t = cpool.tile([128, 128], BF, name="tri_t", tag="tri_t")
            nc.sync.dma_start(tri_t[:], tri.ap())
            id_t = cpool.tile([128, 128], BF, name="id_t", tag="id_t")
            nc.sync.dma_start(id_t[:], ident.ap())

            # v tiles: [t-chunk-row, t-chunk, head, 64 v dims | ones | pad]
            v_t = cpool.tile([128, TC, HEADS_PER_CORE, 66], BF, name="v_t",
                             tag="v_t")
            nc.vector.memset(v_t[:, :, :, 64:65], 1.0)

            # qT/kT per (pair, group), yT2 per pair
            q_t = [[cpool.tile([128, 512], BF, name=f"q_{p}_{g}", tag=f"q_{p}_{g}")
                    for g in range(TG)] for p in range(PAIRS)]
            k_t = [[cpool.tile([128, 512], BF, name=f"k_{p}_{g}", tag=f"k_{p}_{g}")
                    for g in range(TG)] for p in range(PAIRS)]
            yT2_t = [cpool.tile([128, T], BF, name=f"yT2_{p}", tag=f"yT2_{p}")
                     for p in range(PAIRS)]

            # ---------------- qkv generation ----------------
            def emit_v_chunk(tc16):
                psV = ps_u.tile([128, 512], F32, name=f"psV_{tc16}", tag="util")
                for ci in range(CT):
                    nc.tensor.matmul(
                        psV[:],
                        xT_t[:, ci, tc16 * 128:(tc16 + 1) * 128],
                        wv_t[:, ci, :],
                        start=(ci == 0), stop=(ci == CT - 1),
                    )
                nc.vector.tensor_copy(v_t[:, tc16, :, 0:64], psV[:])

            def emit_qk_group(p, g, which):
                w_t, dest = (wq_t, q_t) if which == "q" else (wk_t, k_t)
                ps = ps_u.tile([128, 512], F32, name=f"ps{which}_{p}_{g}",
                               tag="util")
                for ci in range(CT):
                    nc.tensor.matmul(
                        ps[:],
                        w_t[:, ci, p * 128:(p + 1) * 128],
                        xT_t[:, ci, g * 512:(g + 1) * 512],
                        start=(ci == 0), stop=(ci == CT - 1),
                    )
                nc.vector.tensor_copy(dest[p][g][:], ps[:])

            def all_qkv_items(p):
                items = []
                for g in range(TG):
                    items.append(lambda p=p, g=g: emit_qk_group(p, g, "k"))
                    items.append(lambda p=p, g=g: emit_qk_group(p, g, "q"))
                return items

            # ---------------- QK + exp for one (pair, group) ----------------
            # Produces the list of pT chunk tiles for the group.
            def emit_qk_chunk(p, g, c, pT_list):
                jofs = 128 * (c - 4 * g) if c >= 4 * g else 0
                psS = ps_s.tile([128, 2, 512], F32, name=f"psS_{p}_{g}_{c}",
                                tag="s")
                kg, kc = c // 4, c % 4
                for h in range(2):
                    nc.tensor.matmul(
                        psS[:, h, jofs:512],
                        k_t[p][kg][h * 64:(h + 1) * 64, kc * 128:(kc + 1) * 128],
                        q_t[p][g][h * 64:(h + 1) * 64, jofs:512],
                        start=True, stop=True,
                    )
                pT = spool.tile([128, 2, 512], BF, name="pT", tag="pT", bufs=28)
                nc.scalar.activation(pT[:, :, jofs:512], psS[:, :, jofs:512],
                                     AF.Exp, scale=0.125)
                if c >= 4 * g:
                    nc.gpsimd.tensor_mul(
                        pT[:, :, jofs:jofs + 128],
                        pT[:, :, jofs:jofs + 128],
                        tri_t[:, None, :].to_broadcast([128, 2, 128]),
                    )
                pT_list.append(pT)

            # ---------------- flipped att@V for one tq-chunk ----------------
            def emit_attv_tau(p, g, tau, pT_list, yR):
                tg = 4 * g + tau          # global tq chunk index
                psY = ps_y.tile([128, 2, 256], F32, name=f"psY_{p}_{g}_{tau}",
                                tag="y")
                nc.vector.memset(psY[:, :, 0:66], 0.0)
                for c in range(tg + 1):
                    for h in range(2):
                        nc.tensor.matmul(
                            psY[:, h, 0:65],
                            pT_list[c][:, h, tau * 128:(tau + 1) * 128],
                            v_t[:, c, 2 * p + h, 0:65],
                            start=False, stop=(c == tg),
                            skip_group_check=True,
                        )
                nc.gpsimd.tensor_copy(yR[:, tau, :, :], psY[:, :, 0:65])

            # ---------------- per-group normalization + transposes ----------
            def emit_norm(p, g, yR, yN):
                rec = spool.tile([128, TG, 2, 1], F32, name="rec", tag="rec",
                                 bufs=2)
                nc.vector.reciprocal(rec[:], yR[:, :, :, 64:65])
                nc.vector.tensor_mul(
                    yN[:],
                    yR[:, :, :, 0:64],
                    rec[:, :, :, :].to_broadcast([128, TG, 2, 64]),
                )

            def emit_transpose(p, g, tau, yN):
                psT = ps_u.tile([128, 512], F32, name=f"psT_{p}_{g}_{tau}",
                                tag="util")
                psTb = psT[:, 0:64].bitcast(BF)
                nc.tensor.matmul(psTb, yN[:, tau, :, :], id_t[:],
                                 is_transpose=True)
                nc.gpsimd.tensor_copy(
                    yT2_t[p][:, (4 * g + tau) * 128:(4 * g + tau + 1) * 128],
                    psTb,
                )

            # ---------------- projection chunk ----------------
            def emit_proj_chunk(tc16):
                for co2 in range(2):
                    psZ = ps_u.tile([128, 512], F32, name=f"psZ_{tc16}_{co2}",
                                    tag="util")
                    for p in range(PAIRS):
                        nc.tensor.matmul(
                            psZ[:],
                            yT2_t[p][:, tc16 * 128:(tc16 + 1) * 128],
                            wp_t[:, p, co2 * 512:(co2 + 1) * 512],
                            start=(p == 0), stop=(p == PAIRS - 1),
                        )
                    z = spool.tile([128, 512], F32, name="z", tag="z", bufs=4)
                    nc.vector.tensor_copy(z[:], psZ[:])
                    nc.sync.dma_start(
                        y.ap()[tc16 * 128:(tc16 + 1) * 128,
                               co2 * 512:(co2 + 1) * 512],
                        z[:],
                    )

            # ---------------- emission schedule ----------------
            # prologue: v chunks 0..3 + qkv(pair 0)
            for tc16 in range(4):
                emit_v_chunk(tc16)
            for item in all_qkv_items(0):
                item()

            # deferred attention work for the previous group:
            # attV taus + norm + transposes, executed as interleave items
            pending = []   # work items for group (p, g-1)
            fillers = []   # low-priority PE work (qkvgen / proj)

            def pump(n):
                for _ in range(n):
                    if pending:
                        pending.pop(0)()
                    elif fillers:
                        fillers.pop(0)()

            for p in range(PAIRS):
                if p == 0:
                    fillers += [lambda t=t: emit_v_chunk(t) for t in range(4, TC)]
                if p + 1 < PAIRS:
                    fillers += all_qkv_items(p + 1)
                for g in range(TG):
                    nchunks = 4 * g + 4
                    pT_list = []
                    yR = spool.tile([128, TG, 2, 65], F32, name="yR", tag="yR",
                                    bufs=2)
                    yN = spool.tile([128, TG, 2, 64], BF, name="yN", tag="yN",
                                    bufs=3)
                    for c in range(nchunks):
                        emit_qk_chunk(p, g, c, pT_list)
                        # one deferred/filler item per chunk keeps PE fed while
                        # ACT exp paces the group
                        pump(1)
                    # queue this group's attV/norm/transpose work; it runs
                    # interleaved into the next group's chunk loop
                    for tau in range(TG):
                        pending.append(
                            lambda p=p, g=g, tau=tau, pl=pT_list, yR=yR:
                            emit_attv_tau(p, g, tau, pl, yR))
                    pending.append(lambda p=p, g=g, yR=yR, yN=yN:
                                   emit_norm(p, g, yR, yN))
                    for tau in range(TG):
                        pending.append(lambda p=p, g=g, tau=tau, yN=yN:
                                       emit_transpose(p, g, tau, yN))
                    if p == PAIRS - 1 and g >= 1:
                        # pair 3, group g-1 fully transposed -> those proj
                        # chunks are ready
                        fillers += [lambda t=t: emit_proj_chunk(t)
                                    for t in range(4 * (g - 1), 4 * g)]
            # drain
            while pending or fillers:
                pump(1)
            for tc16 in range(12, TC):
                emit_proj_chunk(tc16)

    nc.compile()
    return nc


def _get_compiled():
    global _compiled
    if _compiled is None:
        _compiled = _build()
    return _compiled


def kernel(x, W_attn, W_proj, _trace=False):
    x = np.asarray(x)
    W_attn = np.asarray(W_attn)
    W_proj = np.asarray(W_proj)
    nc = _get_compiled()

    tri = np.triu(np.ones((128, 128), np.float32)).astype(ml_dtypes.bfloat16)
    ident = np.eye(128, dtype=np.float32).astype(ml_dtypes.bfloat16)
    in_maps = []
    for core in range(N_CORES):
        b, hg = core // 2, core % 2
        cols = slice(hg * 512, (hg + 1) * 512)
        in_maps.append({
            "xT": np.ascontiguousarray(x[b].T).astype(ml_dtypes.bfloat16),
            "wq": W_attn[:, 0 * C:1 * C][:, cols].astype(ml_dtypes.bfloat16),
            "wk": W_attn[:, 1 * C:2 * C][:, cols].astype(ml_dtypes.bfloat16),
            "wv": W_attn[:, 2 * C:3 * C][:, cols].astype(ml_dtypes.bfloat16),
            "wp": W_proj[hg * 512:(hg + 1) * 512, :].astype(ml_dtypes.bfloat16),
            "tri": tri,
            "ident": ident,
        })

    res = run_bass_kernel_spmd(nc, in_maps, list(range(N_CORES)), trace=_trace)
    out = np.empty((B, T, C), np.float32)
    for b in range(B):
        out[b] = res.results[2 * b]["y"] + res.results[2 * b + 1]["y"]
    if _trace:
        kernel._last_exec_time_ns = res.exec_time_ns
        kernel._last_results = res
    return out
